# revision 1
# baseline (speedup 1.0000x reference)
"""GTLayer (gnn_message_passing) Trainium2 kernel.

Strategy (8 NeuronCores, SPMD, no collectives):
  * Edges are sharded by destination row: core c owns rows [c*12500, (c+1)*12500).
  * Host sorts each core's edges by destination and greedily packs the
    destination segments, in order, into "groups" capped at 128 segments and
    768 edges (6 tiles of 128).  Each group owns 128 output rows in a
    position-remapped layout; the host un-permutes at the end.  Group count is
    padded to a common g_total so one compiled program serves all cores.
  * Device, per core:
      phase 0: Q_local = embeds_slice @ qTrans   (node-level, own shard only)
      main loop, per 128-edge tile:
        - indirect-DMA gather of [embeds|filt] rows by edge col (544B rows
          padded to 576B) and of Q_local rows by edge dest (512B)
        - PE transpose of gathered cols, one fp32r matmul against
          [kTrans|vTrans] producing ke|ve
        - att = clip(sum_head(q*k), +-10) + filt[col]; expAtt = exp(att)
        - scatter via one-hot matmul accumulated in PSUM:
            acc[seg, 0:128] += expAtt*ve ; acc[seg, 128:132] += expAtt
      per group: res[seg] = acc[seg, 0:128] / (acc[seg, 128:132] + 1e-8)
    The segment-softmax denominator is applied after the scatter (it is
    constant within a segment), so a single pass over edges suffices.
"""

import numpy as np

N = 100000
E = 600000
LATDIM = 128
HEAD = 4
HDIM = LATDIM // HEAD
NCORES = 8
NLOC = N // NCORES              # 12500
CAP_S = 128                     # segments per group (PSUM partition limit)
K_TILES = 6                     # tiles per group
CAP_E = K_TILES * 128           # edge slots per group
PAD_SEG = 999.0
EXT = 144                       # emb_ext row: 128 emb + 4 filt + 12 pad (576B, 64B-aligned)

_CACHE = {}


# --------------------------------------------------------------------------
# host-side planning
# --------------------------------------------------------------------------

def _plan_core(rows, cols, base):
    sel = (rows >= base) & (rows < base + NLOC)
    r = (rows[sel].astype(np.int64) - base)
    c = cols[sel].astype(np.int64)
    o = np.argsort(r, kind="stable")
    r = r[o]
    c = c[o]

    seg_nodes, seg_starts, seg_counts = np.unique(
        r, return_index=True, return_counts=True
    )
    nseg = len(seg_nodes)

    group_bounds = []
    lo = 0
    cur_edges = 0
    for i in range(nseg):
        cnt = int(seg_counts[i])
        if (i - lo) + 1 > CAP_S or cur_edges + cnt > CAP_E:
            group_bounds.append((lo, i))
            lo = i
            cur_edges = 0
        cur_edges += cnt
    group_bounds.append((lo, nseg))

    ngroups = len(group_bounds)
    cidx = np.zeros((ngroups, CAP_E), dtype=np.int32)
    qseg = np.zeros((ngroups, CAP_S), dtype=np.int32)
    segrel = np.full((ngroups, CAP_E), PAD_SEG, dtype=np.float32)
    remap_rows = []
    remap_nodes = []
    for g, (slo, shi) in enumerate(group_bounds):
        e_lo = int(seg_starts[slo])
        e_hi = int(seg_starts[shi]) if shi < nseg else len(r)
        ne = e_hi - e_lo
        cidx[g, :ne] = c[e_lo:e_hi]
        qseg[g, : shi - slo] = seg_nodes[slo:shi]
        segrel[g, :ne] = np.repeat(
            np.arange(shi - slo, dtype=np.float32), seg_counts[slo:shi]
        )
        remap_rows.append(g * CAP_S + np.arange(shi - slo, dtype=np.int64))
        remap_nodes.append(seg_nodes[slo:shi])

    return dict(
        ngroups=ngroups,
        cidx=cidx,
        qseg=qseg,
        segrel=segrel,
        remap_rows=np.concatenate(remap_rows),
        remap_nodes=np.concatenate(remap_nodes),
    )


def _device_layout(arr, g_total, fill):
    """[ngroups, CAP_E] -> [128, g_total*K_TILES] wrapped: edge t*128+p -> (p,t)."""
    g = arr.shape[0]
    out = np.full((128, g_total * K_TILES), fill, dtype=arr.dtype)
    out[:, : g * K_TILES] = arr.reshape(g * K_TILES, 128).T
    return out


# --------------------------------------------------------------------------
# device program
# --------------------------------------------------------------------------

def _build_nc(g_total, nlocp, n_tab):
    import concourse.bass as bass
    import concourse.mybir as mybir
    import concourse.tile as tile
    from concourse import bacc

    f32 = mybir.dt.float32
    f32r = mybir.dt.float32r
    i32 = mybir.dt.int32
    T = g_total * K_TILES

    nc = bacc.Bacc(None, target_bir_lowering=False, debug=True, num_swdge_queues=4)

    emb_ext = nc.dram_tensor("emb_ext", [n_tab, EXT], f32, kind="ExternalInput")
    emb_sl = nc.dram_tensor("emb_sl", [nlocp, LATDIM], f32, kind="ExternalInput")
    kv = nc.dram_tensor("kv", [LATDIM, 2 * LATDIM], f32, kind="ExternalInput")
    qT = nc.dram_tensor("qT", [LATDIM, LATDIM], f32, kind="ExternalInput")
    iota = nc.dram_tensor("iota", [128, 128], f32, kind="ExternalInput")
    iotac = nc.dram_tensor("iotac", [128, 1], f32, kind="ExternalInput")
    ident = nc.dram_tensor("ident", [128, 128], f32, kind="ExternalInput")
    cidx = nc.dram_tensor("cidx", [128, T], i32, kind="ExternalInput")
    qidx = nc.dram_tensor("qidx", [128, g_total], i32, kind="ExternalInput")
    segf = nc.dram_tensor("segf", [128, T], f32, kind="ExternalInput")
    segfr = nc.dram_tensor("segfr", [1, T * 128], f32, kind="ExternalInput")
    res = nc.dram_tensor("res", [g_total * CAP_S, LATDIM], f32, kind="ExternalOutput")
    qloc = nc.dram_tensor("qloc", [nlocp, LATDIM], f32)  # internal scratch

    with tile.TileContext(nc) as tc:
        with (
            tc.tile_pool(name="const", bufs=1) as constp,
            tc.tile_pool(name="gather", bufs=4) as gatherp,
            tc.tile_pool(name="work", bufs=4) as workp,
            tc.tile_pool(name="outp", bufs=3) as outp,
            tc.tile_pool(name="ps", bufs=2, space="PSUM") as psp,
            tc.tile_pool(name="accps", bufs=2, space="PSUM") as accp,
        ):
            # ---- constants ----
            kv_ld = constp.tile([LATDIM, 2 * LATDIM], f32, tag="kv_ld")
            nc.sync.dma_start(kv_ld[:], kv[:])
            kv_sb = constp.tile([LATDIM, 2 * LATDIM], f32r, tag="kv")
            nc.vector.tensor_copy(kv_sb[:], kv_ld[:])
            qT_ld = constp.tile([LATDIM, LATDIM], f32, tag="qT_ld")
            nc.sync.dma_start(qT_ld[:], qT[:])
            qT_sb = constp.tile([LATDIM, LATDIM], f32r, tag="qT")
            nc.vector.tensor_copy(qT_sb[:], qT_ld[:])
            iota_sb = constp.tile([128, 128], f32, tag="iota")
            nc.sync.dma_start(iota_sb[:], iota[:])
            iota_col_sb = constp.tile([128, 1], f32, tag="iotac")
            nc.sync.dma_start(iota_col_sb[:], iotac[:])
            ident_sb = constp.tile([128, 128], f32, tag="ident")
            nc.sync.dma_start(ident_sb[:], ident[:])
            cidx_sb = constp.tile([128, T], i32, tag="cidx")
            nc.sync.dma_start(cidx_sb[:], cidx[:])
            qidx_sb = constp.tile([128, g_total], i32, tag="qidx")
            nc.sync.dma_start(qidx_sb[:], qidx[:])
            segf_sb = constp.tile([128, T], f32, tag="segf")
            nc.sync.dma_start(segf_sb[:], segf[:])

            # ---- phase 0: Q_local = emb_sl @ qTrans ----
            for b in range(nlocp // 128):
                es = workp.tile([128, LATDIM], f32, tag="p0in")
                nc.sync.dma_start(es[:], emb_sl[b * 128 : (b + 1) * 128, :])
                esT_ps = psp.tile([128, LATDIM], f32, tag="tp_ps")
                nc.tensor.transpose(esT_ps[:], es[:], ident_sb[:])
                esT = workp.tile([128, LATDIM], f32r, tag="p0tps")
                nc.scalar.copy(esT[:], esT_ps[:])
                qb_ps = psp.tile([128, LATDIM], f32, tag="mm_ps")
                nc.tensor.matmul(
                    qb_ps[:], esT[:], qT_sb[:], start=True, stop=True
                )
                qb = workp.tile([128, LATDIM], f32, tag="p0out")
                nc.vector.tensor_copy(qb[:], qb_ps[:])
                nc.sync.dma_start(qloc[b * 128 : (b + 1) * 128, :], qb[:])

            # qloc (DRAM) -> gathers below: DRAM deps are not tracked by Tile
            tc.strict_bb_all_engine_barrier()

            # ---- main loop over groups ----
            for g in range(g_total):
                ce = gatherp.tile([128, K_TILES, EXT], f32, tag="ce")
                for t in range(K_TILES):
                    tt = g * K_TILES + t
                    # HW DGE supports one offset per partition per instruction
                    bi = nc.gpsimd.indirect_dma_start(
                        out=ce[:, t, :],
                        out_offset=None,
                        in_=emb_ext[:],
                        in_offset=bass.IndirectOffsetOnAxis(
                            ap=cidx_sb[:, tt : tt + 1], axis=0
                        ),
                    )
                    bi.ins.queue = f"qPoolDynamic{(tt % 4) or ''}"
                # one q row per destination segment; expanded on-chip per tile
                qs = gatherp.tile([128, LATDIM], f32, tag="qs")
                bi = nc.gpsimd.indirect_dma_start(
                    out=qs[:],
                    out_offset=None,
                    in_=qloc[:],
                    in_offset=bass.IndirectOffsetOnAxis(
                        ap=qidx_sb[:, g : g + 1], axis=0
                    ),
                )
                bi.ins.queue = f"qPoolDynamic{(g % 4) or ''}"

                acc_ps = accp.tile([128, LATDIM + HEAD], f32, tag="acc")
                for t in range(K_TILES):
                    tt = g * K_TILES + t
                    # one-hot [e, s] for the scatter; its transpose expands q
                    oh = workp.tile([128, 128], f32r, tag="oh")
                    nc.vector.tensor_tensor(
                        oh[:],
                        segf_sb[:, tt : tt + 1].to_broadcast([128, 128]),
                        iota_sb[:],
                        op=mybir.AluOpType.is_equal,
                    )
                    segb = workp.tile([128, 128], f32, tag="segb")
                    nc.sync.dma_start(
                        segb[:],
                        segfr[0:1, tt * 128 : (tt + 1) * 128].partition_broadcast(128),
                    )
                    ohT = workp.tile([128, 128], f32, tag="ohT")
                    nc.vector.tensor_scalar(
                        ohT[:],
                        segb[:],
                        iota_col_sb[:, 0:1],
                        None,
                        op0=mybir.AluOpType.is_equal,
                    )
                    qe_ps = psp.tile([128, LATDIM], f32, tag="qe_ps")
                    nc.tensor.matmul(qe_ps[:], ohT[:], qs[:], start=True, stop=True)
                    qe_sb = workp.tile([128, LATDIM], f32, tag="qe_sb")
                    nc.vector.tensor_copy(qe_sb[:], qe_ps[:])

                    ceT_ps = psp.tile([128, 128], f32, tag="tp_ps")
                    nc.tensor.transpose(ceT_ps[:], ce[:, t, 0:LATDIM], ident_sb[:])
                    ceT = workp.tile([128, 128], f32r, tag="ceT")
                    nc.vector.tensor_copy(ceT[:], ceT_ps[:])
                    kv_ps = psp.tile([128, 2 * LATDIM], f32, tag="mm_ps")
                    nc.tensor.matmul(
                        kv_ps[:], ceT[:], kv_sb[:], start=True, stop=True
                    )
                    # att = per-head dot(q, k)
                    qk = workp.tile([128, LATDIM], f32, tag="qk")
                    nc.vector.tensor_tensor(
                        qk[:], qe_sb[:], kv_ps[:, 0:LATDIM], op=mybir.AluOpType.mult
                    )
                    att = workp.tile([128, HEAD], f32, tag="att")
                    nc.vector.reduce_sum(
                        att[:].rearrange("p (h o) -> p h o", o=1),
                        qk[:].rearrange("p (h d) -> p h d", h=HEAD),
                        axis=mybir.AxisListType.X,
                    )
                    # clip, + filt
                    nc.vector.tensor_scalar(
                        att[:],
                        att[:],
                        10.0,
                        -10.0,
                        op0=mybir.AluOpType.min,
                        op1=mybir.AluOpType.max,
                    )
                    nc.vector.tensor_add(att[:], att[:], ce[:, t, LATDIM : LATDIM + HEAD])
                    expatt = workp.tile([128, HEAD], f32, tag="expatt")
                    nc.scalar.activation(
                        expatt[:], att[:], mybir.ActivationFunctionType.Exp
                    )
                    # rhs = [expatt*ve | expatt]
                    rhs = workp.tile([128, LATDIM + HEAD], f32r, tag="rhs")
                    nc.vector.tensor_tensor(
                        rhs[:, 0:LATDIM].rearrange("p (h d) -> p h d", h=HEAD),
                        kv_ps[:, LATDIM : 2 * LATDIM].rearrange(
                            "p (h d) -> p h d", h=HEAD
                        ),
                        expatt[:].rearrange("p (h o) -> p h o", o=1).to_broadcast(
                            [128, HEAD, HDIM]
                        ),
                        op=mybir.AluOpType.mult,
                    )
                    nc.vector.tensor_copy(rhs[:, LATDIM : LATDIM + HEAD], expatt[:])
                    nc.tensor.matmul(
                        acc_ps[:],
                        oh[:],
                        rhs[:],
                        start=(t == 0),
                        stop=(t == K_TILES - 1),
                    )

                # normalize and write out
                rn = workp.tile([128, HEAD], f32, tag="rn")
                nc.vector.tensor_scalar_add(rn[:], acc_ps[:, LATDIM : LATDIM + HEAD], 1e-8)
                nc.vector.reciprocal(rn[:], rn[:])
                outb = outp.tile([128, LATDIM], f32, tag="outb")
                for h in range(HEAD):
                    nc.vector.tensor_scalar_mul(
                        outb[:, h * HDIM : (h + 1) * HDIM],
                        acc_ps[:, h * HDIM : (h + 1) * HDIM],
                        rn[:, h : h + 1],
                    )
                nc.sync.dma_start(res[g * CAP_S : (g + 1) * CAP_S, :], outb[:])

    nc.compile()
    return nc


# --------------------------------------------------------------------------
# entry point
# --------------------------------------------------------------------------

def _segfr_layout(segrel, g_total):
    out = np.full((1, g_total * K_TILES * 128), PAD_SEG, dtype=np.float32)
    out[0, : segrel.size] = segrel.reshape(-1)
    return out


def _qseg_layout(qseg, g_total):
    out = np.zeros((128, g_total), dtype=np.int32)
    out[:, : qseg.shape[0]] = qseg.T
    return out


def _prepare(embeds, qTrans, kTrans, vTrans, filt, rows, cols):
    emb_ext = np.zeros((N, EXT), dtype=np.float32)
    emb_ext[:, :LATDIM] = embeds
    emb_ext[:, LATDIM : LATDIM + HEAD] = filt

    plans = [_plan_core(rows, cols, c * NLOC) for c in range(NCORES)]
    g_total = max(p["ngroups"] for p in plans)
    nlocp = ((NLOC + 127) // 128) * 128

    kvw = np.concatenate([kTrans, vTrans], axis=1).astype(np.float32)
    iota = np.tile(np.arange(128, dtype=np.float32), (128, 1))
    ident = np.eye(128, dtype=np.float32)

    in_maps = []
    for c in range(NCORES):
        p = plans[c]
        emb_sl = np.zeros((nlocp, LATDIM), dtype=np.float32)
        emb_sl[:NLOC] = embeds[c * NLOC : (c + 1) * NLOC]
        in_maps.append(
            {
                "emb_ext": emb_ext,
                "emb_sl": emb_sl,
                "kv": kvw,
                "qT": np.ascontiguousarray(qTrans.astype(np.float32)),
                "iota": iota,
                "ident": ident,
                "cidx": _device_layout(p["cidx"], g_total, 0),
                "qidx": _qseg_layout(p["qseg"], g_total),
                "segf": _device_layout(p["segrel"], g_total, PAD_SEG),
                "segfr": _segfr_layout(p["segrel"], g_total),
                "iotac": np.arange(128, dtype=np.float32)[:, None],
            }
        )
    return plans, g_total, nlocp, in_maps


LAST_RESULT = None  # BassKernelResults of the most recent run (for profiling)


def kernel(embeds, qTrans, kTrans, vTrans, filt, rows, cols, _trace=False):
    global LAST_RESULT
    from concourse.bass_utils import run_bass_kernel_spmd

    embeds = np.asarray(embeds, dtype=np.float32)
    qTrans = np.asarray(qTrans, dtype=np.float32)
    kTrans = np.asarray(kTrans, dtype=np.float32)
    vTrans = np.asarray(vTrans, dtype=np.float32)
    filt = np.asarray(filt, dtype=np.float32)
    rows = np.asarray(rows)
    cols = np.asarray(cols)

    plans, g_total, nlocp, in_maps = _prepare(
        embeds, qTrans, kTrans, vTrans, filt, rows, cols
    )

    key = (g_total, nlocp)
    if key not in _CACHE:
        _CACHE[key] = _build_nc(g_total, nlocp, N)
    nc = _CACHE[key]

    import os

    trace = _trace or bool(os.environ.get("GT_TRACE"))
    br = run_bass_kernel_spmd(nc, in_maps, core_ids=list(range(NCORES)), trace=trace)
    LAST_RESULT = br

    out = np.zeros((N, LATDIM), dtype=np.float32)
    for c in range(NCORES):
        p = plans[c]
        dev = br.results[c]["res"]
        out[c * NLOC + p["remap_nodes"]] = dev[p["remap_rows"]]
    return out

